# revision 7
# baseline (speedup 1.0000x reference)
"""Trainium2 Bass kernel: pairwise BiLSTM head/mod scorer (ConcatHeadModule).

Computes scores[i, j] = sum_h v[h] * tanh(A'[i,h] + Bb[j,h]) + outBias where
  A' = tanh(x_i @ W_foh + cb_h) @ hid2Layer[:H] + hid2Bias   (i-shard rows)
  Bb = tanh(x_j @ W_fom + cb_m) @ hid2Layer[H:]              (all j rows)
with n=1024, 2L=512, H=512, H2=256.

Sharding: head axis i split 8 ways (128 rows/core); all weights + full x
replicated per core.  Per core:
  - preamble: PE matmuls produce A'^T [256h x 128i] and Bb^T [256h x 1024j]
    directly transposed (h on partitions, 2 chunks of 128).
  - main loop (per i): DVE outer-add M = Bb^T_c + A'^T_c[:, i] (per-partition
    scalar), ACT tanh on wide [128, 8192] tiles (bottleneck engine),
    PE matvec with lhsT = v chunk (M=1) accumulating scores rows into PSUM
    partitions {0,32,64,96}, DVE copy+outBias to SBUF, strided DMA to DRAM.
"""

import numpy as np

N = 1024          # tokens (head and mod axes)
L2 = 512          # 2*L, BiLSTM concat width
H = 512           # hidden (headfov/modfov width)
H2 = 256          # hidden2 width
NCORES = 8
SHARD = N // NCORES   # 128 head rows per core
P = 128
G = 8             # i-rows per ACT batch
NBLK = SHARD // G  # 16 blocks per core

_CACHE = {}


def _build_nc():
    """Build + compile the per-core Bass module (SPMD: same NEFF, 8 cores)."""
    from contextlib import ExitStack

    import concourse.mybir as mybir
    import concourse.tile as tile
    from concourse import bacc

    fp32 = mybir.dt.float32
    AF = mybir.ActivationFunctionType

    nc = bacc.Bacc("TRN2", debug=False, enable_asserts=False, num_devices=NCORES)

    # All inputs are pre-arranged on host to the exact SBUF image [128, F]
    # (k-chunks of 128 along partitions, chunk-major on the free dim).
    d_xts = nc.dram_tensor("xts", [P, 4 * SHARD], fp32, kind="ExternalInput").ap()
    d_xtf = nc.dram_tensor("xtf", [P, 4 * N], fp32, kind="ExternalInput").ap()
    d_wfoh = nc.dram_tensor("wfoh", [P, 4 * H], fp32, kind="ExternalInput").ap()
    d_wfom = nc.dram_tensor("wfom", [P, 4 * H], fp32, kind="ExternalInput").ap()
    d_h2a = nc.dram_tensor("h2a", [P, 4 * H2], fp32, kind="ExternalInput").ap()
    d_h2b = nc.dram_tensor("h2b", [P, 4 * H2], fp32, kind="ExternalInput").ap()
    d_cbh = nc.dram_tensor("cbh", [P, 4], fp32, kind="ExternalInput").ap()
    d_cbm = nc.dram_tensor("cbm", [P, 4], fp32, kind="ExternalInput").ap()
    d_h2bias = nc.dram_tensor("h2bias", [P, 2], fp32, kind="ExternalInput").ap()
    # v padded to [128, 2*32]: column 32*c holds v chunk c, rest zeros, so the
    # matvec can run as an M=32 matmul that initializes whole psum row-blocks.
    d_v = nc.dram_tensor("vw", [P, 64], fp32, kind="ExternalInput").ap()
    d_ob = nc.dram_tensor("ob", [P, 1], fp32, kind="ExternalInput").ap()
    d_out = nc.dram_tensor("scores", [SHARD, N], fp32, kind="ExternalOutput").ap()

    with tile.TileContext(nc) as tc, ExitStack() as ctx:
        persist = ctx.enter_context(tc.tile_pool(name="persist", bufs=1))
        BbT = persist.tile([P, 2 * N], fp32)        # [128, 2048]: (hc, j)
        ApT = persist.tile([P, 2 * SHARD], fp32)    # [128, 256]:  (hc, i)
        v_sb = persist.tile([P, 64], fp32)
        ob_sb = persist.tile([P, 1], fp32)
        nc.sync.dma_start(v_sb[:, :], d_v)
        nc.sync.dma_start(ob_sb[:, :], d_ob)

        # ---------------- preamble: A'^T and Bb^T ----------------
        with tc.tile_pool(name="pre", bufs=1) as pre, \
             tc.tile_pool(name="pps", bufs=2, space="PSUM") as pps:
            wfoh_sb = pre.tile([P, 4 * H], fp32)
            wfom_sb = pre.tile([P, 4 * H], fp32)
            h2a_sb = pre.tile([P, 4 * H2], fp32)
            h2b_sb = pre.tile([P, 4 * H2], fp32)
            xts_sb = pre.tile([P, 4 * SHARD], fp32)
            xtf_sb = pre.tile([P, 4 * N], fp32)
            cbh_sb = pre.tile([P, 4], fp32)
            cbm_sb = pre.tile([P, 4], fp32)
            h2bias_sb = pre.tile([P, 2], fp32)
            for sb, dr in ((wfoh_sb, d_wfoh), (wfom_sb, d_wfom), (h2a_sb, d_h2a),
                           (h2b_sb, d_h2b), (xts_sb, d_xts), (xtf_sb, d_xtf),
                           (cbh_sb, d_cbh), (cbm_sb, d_cbm), (h2bias_sb, d_h2bias)):
                nc.sync.dma_start(sb[:, :], dr)

            # ah^T = tanh(W_foh^T @ x_shard^T + cb_h)   [512f x 128i]
            ahT = pre.tile([P, H], fp32)  # (ft, i)
            for ft in range(4):
                ps = pps.tile([P, SHARD], fp32, tag="ps_s")
                for kc in range(4):
                    nc.tensor.matmul(
                        ps[:, :],
                        lhsT=wfoh_sb[:, kc * H + ft * P: kc * H + (ft + 1) * P],
                        rhs=xts_sb[:, kc * SHARD: (kc + 1) * SHARD],
                        start=(kc == 0), stop=(kc == 3))
                nc.scalar.activation(ahT[:, ft * P:(ft + 1) * P], ps[:, :],
                                     AF.Tanh, bias=cbh_sb[:, ft:ft + 1])

            # A'^T = hid2Layer[:H]^T @ ah^T + hid2Bias   [256h x 128i]
            for hc in range(2):
                ps = pps.tile([P, SHARD], fp32, tag="ps_s")
                for kc in range(4):
                    nc.tensor.matmul(
                        ps[:, :],
                        lhsT=h2a_sb[:, kc * H2 + hc * P: kc * H2 + (hc + 1) * P],
                        rhs=ahT[:, kc * P:(kc + 1) * P],
                        start=(kc == 0), stop=(kc == 3))
                nc.scalar.activation(ApT[:, hc * SHARD:(hc + 1) * SHARD], ps[:, :],
                                     AF.Identity, bias=h2bias_sb[:, hc:hc + 1])

            # am^T = tanh(W_fom^T @ x^T + cb_m)   [512f x 1024j]
            amT = pre.tile([P, 4 * N], fp32)  # (ft, j)
            for ft in range(4):
                for jh in range(2):
                    ps = pps.tile([P, 512], fp32, tag="ps_b")
                    for kc in range(4):
                        nc.tensor.matmul(
                            ps[:, :],
                            lhsT=wfom_sb[:, kc * H + ft * P: kc * H + (ft + 1) * P],
                            rhs=xtf_sb[:, kc * N + jh * 512: kc * N + (jh + 1) * 512],
                            start=(kc == 0), stop=(kc == 3))
                    nc.scalar.activation(
                        amT[:, ft * N + jh * 512: ft * N + (jh + 1) * 512],
                        ps[:, :], AF.Tanh, bias=cbm_sb[:, ft:ft + 1])

            # Bb^T = hid2Layer[H:]^T @ am^T   [256h x 1024j]
            for hc in range(2):
                for jh in range(2):
                    ps = pps.tile([P, 512], fp32, tag="ps_b")
                    for kc in range(4):
                        nc.tensor.matmul(
                            ps[:, :],
                            lhsT=h2b_sb[:, kc * H2 + hc * P: kc * H2 + (hc + 1) * P],
                            rhs=amT[:, kc * N + jh * 512: kc * N + (jh + 1) * 512],
                            start=(kc == 0), stop=(kc == 3))
                    nc.vector.tensor_copy(
                        BbT[:, hc * N + jh * 512: hc * N + (jh + 1) * 512], ps[:, :])

        # ---------------- main pairwise loop ----------------
        mpool = ctx.enter_context(tc.tile_pool(name="mt", bufs=2))
        zpool = ctx.enter_context(tc.tile_pool(name="zt", bufs=3))
        spool = ctx.enter_context(tc.tile_pool(name="stg", bufs=2))
        mpsum = ctx.enter_context(tc.tile_pool(name="mps", bufs=8, space="PSUM"))

        for ib in range(NBLK):
            zs = []
            for c in range(2):
                mt = mpool.tile([P, G * N], fp32, tag="m")
                for g in range(G):
                    i = ib * G + g
                    nc.vector.tensor_scalar_add(
                        mt[:, g * N:(g + 1) * N],
                        BbT[:, c * N:(c + 1) * N],
                        ApT[:, c * SHARD + i: c * SHARD + i + 1])
                zt = zpool.tile([P, G * N], fp32, tag="z")
                nc.scalar.activation(zt[:, :], mt[:, :], AF.Tanh)
                zs.append(zt)

            # 4 psum banks: (gblock 0/1) x (j-half 0/1); rows at 32*(g%4)
            pst = [mpsum.tile([P, 512], fp32, tag="acc", name=f"acc{qq}")
                   for qq in range(4)]
            for g in range(G):
                q, s = g // 4, 32 * (g % 4)
                for jh in range(2):
                    t = pst[q * 2 + jh]
                    for c in range(2):
                        nc.tensor.matmul(
                            t[s:s + 32, :],
                            lhsT=v_sb[:, c * 32:(c + 1) * 32],
                            rhs=zs[c][:, g * N + jh * 512: g * N + (jh + 1) * 512],
                            start=(c == 0), stop=(c == 1),
                            tile_position=(0, s))

            # psum rows {0,32,64,96} -> staging (+outBias), then scatter to DRAM
            for q in range(2):
                stg = spool.tile([P, N], fp32, tag="s")
                for jh in range(2):
                    nc.vector.tensor_scalar_add(
                        stg[:, jh * 512:(jh + 1) * 512],
                        pst[q * 2 + jh][:, :], ob_sb[:, 0:1])
                r0 = ib * G + q * 4
                nc.sync.dma_start(d_out[r0:r0 + 4, :], stg[0:P:32, :])

    nc.compile()
    return nc


def get_nc():
    if "nc" not in _CACHE:
        _CACHE["nc"] = _build_nc()
    return _CACHE["nc"]


def _chunk_p(a):
    """[c*128, M] -> SBUF image [128, c*M] (chunk-major free dim)."""
    k, m = a.shape
    c = k // P
    return np.ascontiguousarray(
        a.reshape(c, P, m).transpose(1, 0, 2).reshape(P, c * m), dtype=np.float32)


def make_in_maps(inputs):
    lstms0 = np.asarray(inputs["lstms0"], dtype=np.float32)
    lstms1 = np.asarray(inputs["lstms1"], dtype=np.float32)
    w_foh = np.asarray(inputs["W_foh"], dtype=np.float32)
    w_fom = np.asarray(inputs["W_fom"], dtype=np.float32)
    cat_bias = np.asarray(inputs["catBias"], dtype=np.float32)
    hid2 = np.asarray(inputs["hid2Layer"], dtype=np.float32)
    hid2_bias = np.asarray(inputs["hid2Bias"], dtype=np.float32)
    out_layer = np.asarray(inputs["outLayer"], dtype=np.float32)
    out_bias = np.asarray(inputs["outBias"], dtype=np.float32)

    x = np.concatenate([lstms0, lstms1], axis=1)          # [1024, 512]
    xtf = _chunk_p(np.ascontiguousarray(x.T))             # [128, 4096]
    wfoh = _chunk_p(w_foh)
    wfom = _chunk_p(w_fom)
    h2a = _chunk_p(hid2[:H])
    h2b = _chunk_p(hid2[H:])
    cbh = np.ascontiguousarray(cat_bias[0, :H].reshape(4, P).T, dtype=np.float32)
    cbm = np.ascontiguousarray(cat_bias[0, H:].reshape(4, P).T, dtype=np.float32)
    h2bias = np.ascontiguousarray(hid2_bias[0].reshape(2, P).T, dtype=np.float32)
    vw = np.zeros((P, 64), dtype=np.float32)
    vw[:, 0] = out_layer[:P, 0]
    vw[:, 32] = out_layer[P:, 0]
    ob = np.full((P, 1), float(out_bias[0, 0]), dtype=np.float32)

    in_maps = []
    for c in range(NCORES):
        xts = _chunk_p(np.ascontiguousarray(x[c * SHARD:(c + 1) * SHARD].T))
        in_maps.append(dict(xts=xts, xtf=xtf, wfoh=wfoh, wfom=wfom, h2a=h2a,
                            h2b=h2b, cbh=cbh, cbm=cbm, h2bias=h2bias, vw=vw,
                            ob=ob))
    return in_maps


def kernel(**inputs):
    from concourse.bass_utils import run_bass_kernel_spmd

    nc = get_nc()
    in_maps = make_in_maps(inputs)
    res = run_bass_kernel_spmd(nc, in_maps, core_ids=list(range(NCORES)))
    out = np.concatenate([res.results[c]["scores"] for c in range(NCORES)], axis=0)
    return np.ascontiguousarray(out, dtype=np.float32)


# revision 17
# speedup vs baseline: 1.3950x; 1.3950x over previous
"""Trainium2 Bass kernel: pairwise BiLSTM head/mod scorer (ConcatHeadModule).

Computes scores[i, j] = sum_h v[h] * tanh(A'[i,h] + Bb[j,h]) + outBias where
  A' = tanh(x_i @ W_foh + cb_h) @ hid2Layer[:H] + hid2Bias   (i-shard rows)
  Bb = tanh(x_j @ W_fom + cb_m) @ hid2Layer[H:]              (all j rows)
with n=1024, 2L=512, H=512, H2=256.

Sharding: head axis i split 8 ways (128 rows/core); all weights + full x
replicated per core.  Per core:
  - preamble: PE matmuls produce A'^T [256h x 128i] and Bb^T [256h x 1024j]
    directly transposed (h on partitions, 2 chunks of 128).
  - main loop (per i): DVE outer-add M = Bb^T_c + A'^T_c[:, i] (per-partition
    scalar), ACT tanh on wide [128, 8192] tiles (bottleneck engine),
    PE matvec with lhsT = v chunk (M=1) accumulating scores rows into PSUM
    partitions {0,32,64,96}, DVE copy+outBias to SBUF, strided DMA to DRAM.
"""

import numpy as np

N = 1024          # tokens (head and mod axes)
L2 = 512          # 2*L, BiLSTM concat width
H = 512           # hidden (headfov/modfov width)
H2 = 256          # hidden2 width
NCORES = 8
SHARD = N // NCORES   # 128 head rows per core
P = 128
G = 8             # i-rows per ACT batch
NBLK = SHARD // G  # 16 blocks per core

_CACHE = {}


def _build_nc():
    """Build + compile the per-core Bass module (SPMD: same NEFF, 8 cores)."""
    from contextlib import ExitStack

    import concourse.mybir as mybir
    import concourse.tile as tile
    from concourse import bacc

    fp32 = mybir.dt.float32
    bf16 = mybir.dt.bfloat16
    AF = mybir.ActivationFunctionType

    nc = bacc.Bacc("TRN2", debug=False, enable_asserts=False, num_devices=NCORES)

    # All inputs are pre-arranged on host to the exact SBUF image [128, F]
    # (k-chunks of 128 along partitions, chunk-major on the free dim).
    d_xts = nc.dram_tensor("xts", [P, 4 * SHARD], fp32, kind="ExternalInput").ap()
    d_xtf = nc.dram_tensor("xtf", [P, 4 * N], fp32, kind="ExternalInput").ap()
    d_wfoh = nc.dram_tensor("wfoh", [P, 4 * H], fp32, kind="ExternalInput").ap()
    d_wfom = nc.dram_tensor("wfom", [P, 4 * H], fp32, kind="ExternalInput").ap()
    d_h2a = nc.dram_tensor("h2a", [P, 4 * H2], fp32, kind="ExternalInput").ap()
    d_h2b = nc.dram_tensor("h2b", [P, 4 * H2], fp32, kind="ExternalInput").ap()
    d_cbh = nc.dram_tensor("cbh", [P, 4], fp32, kind="ExternalInput").ap()
    d_cbm = nc.dram_tensor("cbm", [P, 4], fp32, kind="ExternalInput").ap()
    d_h2bias = nc.dram_tensor("h2bias", [P, 2], fp32, kind="ExternalInput").ap()
    # v padded to [128, 2*32]: column 32*c holds v chunk c, rest zeros, so the
    # matvec can run as an M=32 matmul that initializes whole psum row-blocks.
    d_v = nc.dram_tensor("vw", [P, 64], fp32, kind="ExternalInput").ap()
    d_ob = nc.dram_tensor("ob", [P, 1], fp32, kind="ExternalInput").ap()
    d_out = nc.dram_tensor("scores", [SHARD, N], fp32, kind="ExternalOutput").ap()

    with tile.TileContext(nc) as tc, ExitStack() as ctx:
        persist = ctx.enter_context(tc.tile_pool(name="persist", bufs=1))
        BbT = persist.tile([P, 2 * N], fp32)        # [128, 2048]: (hc, j)
        ApT = persist.tile([P, 2 * SHARD], fp32)    # [128, 256]:  (hc, i)
        v_sb = persist.tile([P, 64], fp32)
        v_bf = persist.tile([P, 64], bf16)
        ob_sb = persist.tile([P, 1], fp32)
        nc.sync.dma_start(v_sb[:, :], d_v)
        nc.sync.dma_start(ob_sb[:, :], d_ob)
        nc.vector.tensor_copy(v_bf[:, :], v_sb[:, :])

        # ---------------- preamble: A'^T and Bb^T ----------------
        with tc.tile_pool(name="pre", bufs=1) as pre, \
             tc.tile_pool(name="pps", bufs=2, space="PSUM") as pps:
            wfoh_sb = pre.tile([P, 4 * H], fp32)
            wfom_sb = pre.tile([P, 4 * H], fp32)
            h2a_sb = pre.tile([P, 4 * H2], fp32)
            h2b_sb = pre.tile([P, 4 * H2], fp32)
            xts_sb = pre.tile([P, 4 * SHARD], fp32)
            xtf_sb = pre.tile([P, 4 * N], fp32)
            cbh_sb = pre.tile([P, 4], fp32)
            cbm_sb = pre.tile([P, 4], fp32)
            h2bias_sb = pre.tile([P, 2], fp32)
            for sb, dr in ((wfoh_sb, d_wfoh), (wfom_sb, d_wfom), (h2a_sb, d_h2a),
                           (h2b_sb, d_h2b), (xts_sb, d_xts), (xtf_sb, d_xtf),
                           (cbh_sb, d_cbh), (cbm_sb, d_cbm), (h2bias_sb, d_h2bias)):
                nc.sync.dma_start(sb[:, :], dr)

            # ah^T = tanh(W_foh^T @ x_shard^T + cb_h)   [512f x 128i]
            ahT = pre.tile([P, H], fp32)  # (ft, i)
            for ft in range(4):
                ps = pps.tile([P, SHARD], fp32, tag="ps_s")
                for kc in range(4):
                    nc.tensor.matmul(
                        ps[:, :],
                        lhsT=wfoh_sb[:, kc * H + ft * P: kc * H + (ft + 1) * P],
                        rhs=xts_sb[:, kc * SHARD: (kc + 1) * SHARD],
                        start=(kc == 0), stop=(kc == 3))
                nc.scalar.activation(ahT[:, ft * P:(ft + 1) * P], ps[:, :],
                                     AF.Tanh, bias=cbh_sb[:, ft:ft + 1])

            # A'^T = hid2Layer[:H]^T @ ah^T + hid2Bias   [256h x 128i]
            for hc in range(2):
                ps = pps.tile([P, SHARD], fp32, tag="ps_s")
                for kc in range(4):
                    nc.tensor.matmul(
                        ps[:, :],
                        lhsT=h2a_sb[:, kc * H2 + hc * P: kc * H2 + (hc + 1) * P],
                        rhs=ahT[:, kc * P:(kc + 1) * P],
                        start=(kc == 0), stop=(kc == 3))
                nc.scalar.activation(ApT[:, hc * SHARD:(hc + 1) * SHARD], ps[:, :],
                                     AF.Identity, bias=h2bias_sb[:, hc:hc + 1])

            # am^T = tanh(W_fom^T @ x^T + cb_m)   [512f x 1024j]
            amT = pre.tile([P, 4 * N], fp32)  # (ft, j)
            for ft in range(4):
                for jh in range(2):
                    ps = pps.tile([P, 512], fp32, tag="ps_b")
                    for kc in range(4):
                        nc.tensor.matmul(
                            ps[:, :],
                            lhsT=wfom_sb[:, kc * H + ft * P: kc * H + (ft + 1) * P],
                            rhs=xtf_sb[:, kc * N + jh * 512: kc * N + (jh + 1) * 512],
                            start=(kc == 0), stop=(kc == 3))
                    nc.scalar.activation(
                        amT[:, ft * N + jh * 512: ft * N + (jh + 1) * 512],
                        ps[:, :], AF.Tanh, bias=cbm_sb[:, ft:ft + 1])

            # Bb^T = hid2Layer[H:]^T @ am^T   [256h x 1024j]
            for hc in range(2):
                for jh in range(2):
                    ps = pps.tile([P, 512], fp32, tag="ps_b")
                    for kc in range(4):
                        nc.tensor.matmul(
                            ps[:, :],
                            lhsT=h2b_sb[:, kc * H2 + hc * P: kc * H2 + (hc + 1) * P],
                            rhs=amT[:, kc * N + jh * 512: kc * N + (jh + 1) * 512],
                            start=(kc == 0), stop=(kc == 3))
                    nc.vector.tensor_copy(
                        BbT[:, hc * N + jh * 512: hc * N + (jh + 1) * 512], ps[:, :])

        # ---------------- main pairwise loop ----------------
        mpool = ctx.enter_context(tc.tile_pool(name="mt", bufs=2))
        zpool = ctx.enter_context(tc.tile_pool(name="zt", bufs=3))
        spool = ctx.enter_context(tc.tile_pool(name="stg", bufs=2))
        mpsum = ctx.enter_context(tc.tile_pool(name="mps", bufs=8, space="PSUM"))

        def emit_matvec(ib, zs):
            # 4 psum banks: (gblock 0/1) x (j-half 0/1); rows at 32*(g%4)
            pst = [mpsum.tile([P, 512], fp32, tag="acc", name=f"acc{qq}")
                   for qq in range(4)]
            for g in range(G):
                q, s = g // 4, 32 * (g % 4)
                for jh in range(2):
                    t = pst[q * 2 + jh]
                    for c in range(2):
                        # bf16: 1 cycle/row on PE (fp32 pays 4) — the v-dot
                        # tolerates bf16 rounding (~2e-3 on final scores).
                        nc.tensor.matmul(
                            t[s:s + 32, :],
                            lhsT=v_bf[:, c * 32:(c + 1) * 32],
                            rhs=zs[c][:, g * N + jh * 512: g * N + (jh + 1) * 512],
                            start=(c == 0), stop=(c == 1),
                            tile_position=(0, s))

            # psum rows {0,32,64,96} -> staging (+outBias), then scatter to DRAM
            for q in range(2):
                stg = spool.tile([P, N], fp32, tag="s", name=f"stg{q}")
                for jh in range(2):
                    nc.vector.tensor_scalar_add(
                        stg[:, jh * 512:(jh + 1) * 512],
                        pst[q * 2 + jh][:, :], ob_sb[:, 0:1])
                r0 = ib * G + q * 4
                nc.sync.dma_start(d_out[r0:r0 + 4, :], stg[0:P:32, :])

        # Software-pipelined one block deep: the matvec/staging for block
        # ib-1 is emitted after block ib's adds+tanh, so the DVE never queues
        # a PSUM-draining op ahead of the next block's adds (that ordering
        # serializes every engine per block).
        prev = None
        for ib in range(NBLK):
            zs = []
            for c in range(2):
                mt = mpool.tile([P, G * N], fp32, tag="m", name=f"m{c}")
                for g in range(G):
                    i = ib * G + g
                    nc.vector.tensor_scalar_add(
                        mt[:, g * N:(g + 1) * N],
                        BbT[:, c * N:(c + 1) * N],
                        ApT[:, c * SHARD + i: c * SHARD + i + 1])
                zt = zpool.tile([P, G * N], bf16, tag="z", name=f"z{c}")
                nc.scalar.activation(zt[:, :], mt[:, :], AF.Tanh)
                zs.append(zt)
            if prev is not None:
                emit_matvec(prev[0], prev[1])
            prev = (ib, zs)
        emit_matvec(prev[0], prev[1])

    nc.compile()
    return nc


def get_nc():
    if "nc" not in _CACHE:
        _CACHE["nc"] = _build_nc()
    return _CACHE["nc"]


def _chunk_p(a):
    """[c*128, M] -> SBUF image [128, c*M] (chunk-major free dim)."""
    k, m = a.shape
    c = k // P
    return np.ascontiguousarray(
        a.reshape(c, P, m).transpose(1, 0, 2).reshape(P, c * m), dtype=np.float32)


def make_in_maps(inputs):
    lstms0 = np.asarray(inputs["lstms0"], dtype=np.float32)
    lstms1 = np.asarray(inputs["lstms1"], dtype=np.float32)
    w_foh = np.asarray(inputs["W_foh"], dtype=np.float32)
    w_fom = np.asarray(inputs["W_fom"], dtype=np.float32)
    cat_bias = np.asarray(inputs["catBias"], dtype=np.float32)
    hid2 = np.asarray(inputs["hid2Layer"], dtype=np.float32)
    hid2_bias = np.asarray(inputs["hid2Bias"], dtype=np.float32)
    out_layer = np.asarray(inputs["outLayer"], dtype=np.float32)
    out_bias = np.asarray(inputs["outBias"], dtype=np.float32)

    x = np.concatenate([lstms0, lstms1], axis=1)          # [1024, 512]
    xtf = _chunk_p(np.ascontiguousarray(x.T))             # [128, 4096]
    wfoh = _chunk_p(w_foh)
    wfom = _chunk_p(w_fom)
    h2a = _chunk_p(hid2[:H])
    h2b = _chunk_p(hid2[H:])
    cbh = np.ascontiguousarray(cat_bias[0, :H].reshape(4, P).T, dtype=np.float32)
    cbm = np.ascontiguousarray(cat_bias[0, H:].reshape(4, P).T, dtype=np.float32)
    h2bias = np.ascontiguousarray(hid2_bias[0].reshape(2, P).T, dtype=np.float32)
    vw = np.zeros((P, 64), dtype=np.float32)
    vw[:, 0] = out_layer[:P, 0]
    vw[:, 32] = out_layer[P:, 0]
    ob = np.full((P, 1), float(out_bias[0, 0]), dtype=np.float32)

    in_maps = []
    for c in range(NCORES):
        xts = _chunk_p(np.ascontiguousarray(x[c * SHARD:(c + 1) * SHARD].T))
        in_maps.append(dict(xts=xts, xtf=xtf, wfoh=wfoh, wfom=wfom, h2a=h2a,
                            h2b=h2b, cbh=cbh, cbm=cbm, h2bias=h2bias, vw=vw,
                            ob=ob))
    return in_maps


def kernel(**inputs):
    from concourse.bass_utils import run_bass_kernel_spmd

    nc = get_nc()
    in_maps = make_in_maps(inputs)
    res = run_bass_kernel_spmd(nc, in_maps, core_ids=list(range(NCORES)))
    out = np.concatenate([res.results[c]["scores"] for c in range(NCORES)], axis=0)
    return np.ascontiguousarray(out, dtype=np.float32)
